# revision 18
# baseline (speedup 1.0000x reference)
"""Trainium2 Bass kernel for nn_Decoder_60232621359478 (dense MoE decoder).

Model (per token): 3-layer gating MLP -> softmax over E=8 experts (dense
weights, all experts active), then 4 MoE layers where each layer is
  y = sum_e ew_e * ([z; x] @ W_e + b_e),  x <- elu(y) (except last layer).

Kernel strategy:
- Data-parallel over batch across 8 NeuronCores (B=32 -> 4 per core,
  1024 tokens/core). No collectives.
- Everything on-chip is FEATURE-MAJOR (features on partitions, tokens on
  the free axis), so layer outputs (PSUM is [Dout, tokens]) feed the next
  layer with no transposes. Host pre-transposes the inputs (free numpy).
- Expert gating folded into the matmul contraction:
    sum_e ew_e * (x @ W_e) = concat_e(ew_e * x) @ stack_e(W_e)
  Scaled inputs (ew_e * x) are produced by DVE right before use; each MoE
  layer is one PSUM-accumulated chain of 8 experts x k-tiles.
- All matmul operands are fp16 (PSUM accumulation fp32): same PE column
  rate as float32r but FastWeightLoad-eligible, half the DVE cost and
  weight DMA, and ~8x finer mantissa than bf16 so the compounded
  per-stage rounding stays ~1e-3 vs the 2e-2 gate. (fp8+DoubleRow was
  measured at rel 4.1e-2 on HW -- quantization noise, not worth it; bf16
  measured 1.1e-2.)
- ew-scaled z (ewz[e]) is computed once in layer 0 and reused by all four
  MoE layers (z is 2 of the 6 k-tiles of every layer).
- Layer 0's bias and the 3-row v_hip input block of all 8 experts are
  packed into a single K=32 stationary tile (rows = [bias_e; Wv_e]); its
  moving operand [ew8; ew_e*v] is built with one selector matmul + one
  DVE multiply per chunk. Kills 8 tiny v-matmuls + separate bias matmuls.
- Token-chunk (c) is the OUTER loop everywhere so chunk 0's
  PSUM->elu->next-layer chain overlaps chunk 1's matmuls: the PE never
  idles long enough for the HAM clock gate to re-throttle (cold clock
  costs 2x).
- Softmax over the 8 experts (partition axis) is done with PE tricks
  (colsum via ones(8,1) matmul, row-broadcasts via one-hot stationary
  matmuls, reciprocal on DVE), pipelined in 4 sub-chunks of 256 tokens
  with the 8 per-expert broadcast matmuls issued BEFORE the
  recip-dependent one, and the 8 ewb multiplies run on the otherwise-idle
  GPSIMD engine so the DVE reciprocals don't stall the PE.
- elu(x)+1 is used as the carried activation (elu+1 = exp(min(x,0)) +
  max(x,0): 2 ACT ops + 1 DVE op); the "-1" is folded into the next
  layer's bias on the host (b' = b - colsum(W_xpart), colsum taken over
  the fp16-quantized weights so the fold matches what the chip computes).
- Input DMAs are issued in first-use order with z and the G0 weight split
  into k-tile-sized pieces, so the first matmul only waits for ~0.4MB.
"""

import numpy as np

import concourse.bass as bass
import concourse.mybir as mybir
import concourse.tile as tile
from concourse import bacc
from concourse import bass_utils

dt = mybir.dt
AF = mybir.ActivationFunctionType
ALU = mybir.AluOpType

B, T = 32, 256
DM, DL, DH, DP, E = 256, 256, 512, 16, 8
NCORES = 8
BP = B // NCORES            # batches per core
NT = BP * T                 # tokens per core (1024)
CH = 2                      # token chunks
CT = NT // CH               # tokens per chunk (512)
SC = 4                      # softmax sub-chunks
SCT = NT // SC              # tokens per softmax sub-chunk (256)

_CACHE = {}
F16 = np.float16


def _prep_weights(gw0, gb0, gw1, gb1, gw2, gb2,
                  w0, b0, w1, b1, w2, b2, wo, bo):
    f = np.float32
    # gating: k-tiles [z0, z1, extra]; extra rows 0:16 = p-part, row 16 = bias
    G0 = np.zeros((3, 128, DH), f)
    G0[0] = gw0[0:128]
    G0[1] = gw0[128:256]
    G0[2, 0:16] = gw0[256:272]
    G0[2, 16] = gb0

    def g_later(gw, gb, dout):
        gwq = gw.astype(F16).astype(f)
        Gt = np.zeros((7, 128, dout), f)
        Gt[0:6] = gw[0:768].reshape(6, 128, dout)
        Gt[6, 16] = gb - gwq[256:768].sum(axis=0)  # h' = elu+1 correction
        return Gt

    G1 = g_later(gw1, gb1, DH)
    G2 = g_later(gw2, gb2, E)

    # L0 weight tiles: kt 0-1 = z-part, kt 2-3 = x_curr part
    W0 = np.zeros((E, 4, 128, DH), f)
    W0[:, 0] = w0[:, 0:128]
    W0[:, 1] = w0[:, 128:256]
    W0[:, 2] = w0[:, 259:387]
    W0[:, 3] = w0[:, 387:515]
    # packed K=32 stationary: rows 0-7 = bias_e, rows 8+3e+i = Wv_e[i]
    PK0 = np.zeros((32, DH), f)
    PK0[0:8] = b0
    for e in range(E):
        PK0[8 + 3 * e: 11 + 3 * e] = w0[e, 256:259]

    def moe_later(w, b):
        wq = w.astype(F16).astype(f)
        Wt = np.ascontiguousarray(w.reshape(E, 6, 128, -1)).astype(F16)
        Bt = (b - wq[:, 256:768, :].sum(axis=1)).astype(F16)
        return Wt, Bt

    W1, B1 = moe_later(w1, b1)
    W2, B2 = moe_later(w2, b2)
    WO, BO = moe_later(wo, bo)

    ONES = np.ones((E, 128), f)
    EMAT = np.zeros((E, E * 128), f)
    for e in range(E):
        EMAT[e, e * 128:(e + 1) * 128] = 1.0
    # selector for the packed moving operand: col j<8 -> ew_j ; col 8+3e+i -> ew_e
    EM32 = np.zeros((E, 32), f)
    for e in range(E):
        EM32[e, e] = 1.0
        EM32[e, 8 + 3 * e: 11 + 3 * e] = 1.0
    return dict(G0=G0.astype(F16), G1=G1.astype(F16), G2=G2.astype(F16),
                W0=W0.astype(F16), PK0=PK0.astype(F16),
                W1=W1, B1=B1, W2=W2, B2=B2, WO=WO, BO=BO,
                ONES=ONES.astype(F16), EMAT=EMAT.astype(F16),
                EM32=EM32.astype(F16))


def _prep_core_inputs(z, p_next, v_hip_next, x_curr, core):
    f = np.float32
    sl = slice(core * BP, (core + 1) * BP)
    zT = np.ascontiguousarray(z[sl].reshape(NT, DL).T).astype(F16)
    xcT = np.ascontiguousarray(x_curr[sl].reshape(NT, DM).T).astype(F16)
    gex = np.zeros((128, NT), f)
    gex[0:16] = p_next[sl].reshape(NT, DP).T
    gex[16] = 1.0
    # packed-moving source: rows 0-7 = 1.0 (passes ew8 through), rows
    # 8+3e+i = v_i (so row * selected ew_e = ew_e * v_i)
    v32 = np.ones((32, NT), f)
    vT = v_hip_next[sl].reshape(NT, 3).T
    for e in range(E):
        v32[8 + 3 * e: 11 + 3 * e] = vT
    return dict(zT=zT, xcT=xcT, gex=gex.astype(F16), v32=v32.astype(F16))


def _build():
    nc = bacc.Bacc("TRN2", target_bir_lowering=False, debug=False,
                   num_devices=NCORES)
    f16 = dt.float16
    f32 = dt.float32

    def din(name, shape):
        return nc.dram_tensor(name, shape, f16, kind="ExternalInput").ap()

    zT_d = din("zT", (DL, NT))
    gex_d = din("gex", (128, NT))
    G0_d = din("G0", (3, 128, DH))
    G1_d = din("G1", (7, 128, DH))
    G2_d = din("G2", (7, 128, E))
    xcT_d = din("xcT", (DM, NT))
    v32_d = din("v32", (32, NT))
    ones_d = din("ONES", (E, 128))
    emat_d = din("EMAT", (E, E * 128))
    em32_d = din("EM32", (E, 32))
    W0_d = din("W0", (E, 4, 128, DH))
    PK0_d = din("PK0", (32, DH))
    W1_d = din("W1", (E, 6, 128, DH))
    W2_d = din("W2", (E, 6, 128, DH))
    WO_d = din("WO", (E, 6, 128, DM))
    B1_d = din("B1", (E, DH))
    B2_d = din("B2", (E, DH))
    BO_d = din("BO", (E, DM))
    yT_d = nc.dram_tensor("yT", (DM, NT), f16, kind="ExternalOutput").ap()

    with tile.TileContext(nc) as tc, \
         nc.allow_low_precision(reason="fp16 matmul rounding intended"):
        with tc.tile_pool(name="inp", bufs=1) as inp, \
             tc.tile_pool(name="wp", bufs=8) as wp, \
             tc.tile_pool(name="act", bufs=1) as act, \
             tc.tile_pool(name="xsp", bufs=8) as xsp, \
             tc.tile_pool(name="tmp", bufs=4) as tmpp, \
             tc.tile_pool(name="ps", bufs=8, space="PSUM") as ps:

            # ---- persistent inputs, issued in first-use order; z and G0
            # split per k-tile so matmul #0 waits for ~0.4MB, not 1.2MB ----
            zk = [inp.tile([128, NT], f16, name=f"zk{k}") for k in range(2)]
            g0k = [inp.tile([128, DH], f16, name=f"g0k{k}") for k in range(3)]
            nc.sync.dma_start(zk[0], zT_d[0:128, :])
            nc.sync.dma_start(g0k[0], G0_d[0])
            nc.sync.dma_start(zk[1], zT_d[128:256, :])
            nc.sync.dma_start(g0k[1], G0_d[1])
            gex_sb = inp.tile([128, NT], f16, name="gex_sb")
            nc.sync.dma_start(gex_sb, gex_d)
            nc.sync.dma_start(g0k[2], G0_d[2])
            g1_sb = inp.tile([128, 7, DH], f16, name="g1_sb")
            nc.sync.dma_start(g1_sb, G1_d.rearrange("k p d -> p k d"))
            g2_sb = inp.tile([128, 7, E], f16, name="g2_sb")
            nc.sync.dma_start(g2_sb, G2_d.rearrange("k p d -> p k d"))
            ones_sb = inp.tile([E, 128], f16, name="ones_sb")
            nc.sync.dma_start(ones_sb, ones_d)
            emat_sb = inp.tile([E, E * 128], f16, name="emat_sb")
            nc.sync.dma_start(emat_sb, emat_d)
            em32_sb = inp.tile([E, 32], f16, name="em32_sb")
            nc.sync.dma_start(em32_sb, em32_d)
            xc_sb = inp.tile([128, 2, NT], f16, name="xc_sb")
            nc.sync.dma_start(xc_sb, xcT_d.rearrange("(k p) t -> p k t", p=128))
            v32_sb = inp.tile([32, NT], f16, name="v32_sb")
            nc.sync.dma_start(v32_sb, v32_d)
            pk0_sb = inp.tile([32, DH], f16, name="pk0_sb")
            nc.sync.dma_start(pk0_sb, PK0_d)
            bias_sb = []
            for i, (bd, dout) in enumerate(
                    [(B1_d, DH), (B2_d, DH), (BO_d, DM)]):
                bt = inp.tile([E, dout], f16, name=f"b{i}_sb")
                nc.sync.dma_start(bt, bd)
                bias_sb.append(bt)

            elu_scratch = [
                (tmpp.tile([128, CT], f32, name=f"mn{i}", tag="mn"),
                 tmpp.tile([128, CT], f32, name=f"ex{i}", tag="ex"))
                for i in range(4)]
            elu_ctr = [0]

            def elu_p1(dst, psum):
                """dst = elu(psum) + 1 = exp(min(psum,0)) + max(psum,0).

                min(x,0) = -relu(-x); both unary steps run on ACT so the
                DVE (busy producing scaled inputs) only pays one op. The 4
                scratch pairs are preallocated and cycled (fewer tile
                allocations = less semaphore teardown in the epilogue).
                """
                mn, ex = elu_scratch[elu_ctr[0] % 4]
                elu_ctr[0] += 1
                n = psum.shape[-1]
                p = psum.shape[0]
                nc.scalar.activation(mn[:p, :n], psum, AF.Relu, scale=-1.0)
                nc.scalar.activation(ex[:p, :n], mn[:p, :n], AF.Exp,
                                     scale=-1.0)
                nc.vector.scalar_tensor_tensor(
                    dst, psum, 0.0, ex[:p, :n], ALU.max, ALU.add)

            # ---- gating MLP (chunk-outer so elu chains overlap matmuls) ----
            def glayer(w_of, ktiles, rhs_of, douts, dst_of, kt_order=None):
                order = list(kt_order) if kt_order else list(range(ktiles))
                for c in range(CH):
                    cs = slice(c * CT, (c + 1) * CT)
                    psums = [ps.tile([128, CT], f32, name=f"gps{m}_{c}",
                                     tag="ps") for m in range(douts)]
                    for kt in order:
                        rhs = rhs_of(kt, cs)
                        for m in range(douts):
                            nc.tensor.matmul(
                                psums[m][:, :] if douts > 1
                                else psums[m][:E, :],
                                w_of(kt, m),
                                rhs,
                                start=(kt == order[0]),
                                stop=(kt == order[-1]))
                    dst_of(c, cs, psums)

            h0 = [act.tile([128, NT], f16, name=f"h0_{m}", tag="xp", bufs=8)
                  for m in range(4)]

            def dst_h0(c, cs, psums):
                for m in range(4):
                    elu_p1(h0[m][:, cs], psums[m][:, :])

            glayer(lambda kt, m: g0k[kt][:, m * 128:(m + 1) * 128], 3,
                   lambda kt, cs: (zk[kt][:, cs] if kt < 2
                                   else gex_sb[:, cs]),
                   4, dst_h0)

            h1 = [act.tile([128, NT], f16, name=f"h1_{m}", tag="xp", bufs=8)
                  for m in range(4)]

            def rhs_g1(kt, cs):
                if kt < 2:
                    return zk[kt][:, cs]
                if kt < 6:
                    return h0[kt - 2][:, cs]
                return gex_sb[:, cs]

            def dst_h1(c, cs, psums):
                for m in range(4):
                    elu_p1(h1[m][:, cs], psums[m][:, :])

            glayer(lambda kt, m: g1_sb[:, kt, m * 128:(m + 1) * 128], 7,
                   rhs_g1, 4, dst_h1, kt_order=[0, 1, 6, 2, 3, 4, 5])

            exp_g = act.tile([E, NT], f16, name="exp_g", tag="eg")

            def rhs_g2(kt, cs):
                if kt < 2:
                    return zk[kt][:, cs]
                if kt < 6:
                    return h1[kt - 2][:, cs]
                return gex_sb[:, cs]

            def dst_g2(c, cs, psums):
                nc.scalar.activation(exp_g[:, cs], psums[0][:E, :], AF.Exp)

            glayer(lambda kt, m: g2_sb[:, kt, :], 7,
                   rhs_g2, 1, dst_g2, kt_order=[0, 1, 6, 2, 3, 4, 5])

            # ---- softmax normalization (partition axis, via PE),
            # pipelined in SC sub-chunks. Per sub-chunk: colsum matmul ->
            # reciprocal (DVE) -> tiny [8,SCT] recip broadcast -> normalized
            # ew8 (DVE) -> per-expert [128,SCT] broadcast matmuls of the
            # ALREADY-NORMALIZED ew8, landed with plain copies split across
            # ACT and GPSIMD. Each psum tile releases right after its copy,
            # so the 8-bank ring never cycles, and ewb is an exact copy of
            # ew8 (one rounding stage fewer). ----
            recip = act.tile([1, NT], f16, name="recip", tag="rc")
            scol = act.tile([1, NT], f32, name="scol", tag="sc")
            rbc8 = act.tile([E, NT], f32, name="rbc8", tag="rbc")
            ew8 = act.tile([E, NT], f16, name="ew8", tag="ew8")
            ewb = [act.tile([128, NT], f16, name=f"ewb{e}", tag="ewb", bufs=8)
                   for e in range(E)]
            # phase 1: all 4 colsums up front; each is copied out of PSUM
            # immediately (bank frees fast) and the reciprocal reads the
            # SBUF copy, so the serial recip chain holds no PSUM slots
            for c in range(SC):
                cs = slice(c * SCT, (c + 1) * SCT)
                s_ps = ps.tile([1, SCT], f32, name="s_ps", tag="ps")
                nc.tensor.matmul(s_ps[:, :], ones_sb[:, 0:1], exp_g[:, cs],
                                 start=True, stop=True)
                nc.scalar.copy(scol[:, cs], s_ps[:, :])
                nc.vector.reciprocal(recip[:, cs], scol[:, cs])
            # keep-warm: the PE would otherwise idle ~5us here waiting on
            # the DVE reciprocal chain, and the HAM clock gate would halve
            # the PE clock for the next ~27us (measured). Throwaway colsum
            # matmuls cap every idle stretch below the 3.4us re-throttle
            # window; they cost ~2.6us at full clock vs ~14us of half-clock.
            for i in range(12):
                kw_ps = ps.tile([1, CT], f32, name="kw_ps", tag="ps")
                nc.tensor.matmul(kw_ps[:, :], ones_sb[:, 0:1],
                                 exp_g[:, (i % 2) * CT:(i % 2 + 1) * CT],
                                 start=True, stop=True)

            # phase 2: per sub-chunk, the recip broadcast + 8 expert
            # broadcasts of the ALREADY-NORMALIZED ew8; sub-chunk c's eb
            # matmuls overlap sub-chunk c+1's reciprocal on DVE, so PE
            # gaps stay well under the 3.4us HAM re-throttle window
            for c in range(SC):
                cs = slice(c * SCT, (c + 1) * SCT)
                rb_ps = ps.tile([E, SCT], f32, name="rb_ps", tag="ps")
                nc.tensor.matmul(rb_ps[:, :], ones_sb[0:1, 0:E],
                                 recip[:, cs], start=True, stop=True)
                nc.scalar.copy(rbc8[:, cs], rb_ps[:, :])
                nc.vector.tensor_mul(ew8[:, cs], exp_g[:, cs], rbc8[:, cs])
                for e in range(E):
                    eb_ps = ps.tile([128, SCT], f32, name="eb_ps", tag="ps")
                    nc.tensor.matmul(
                        eb_ps[:, :], emat_sb[:, e * 128:(e + 1) * 128],
                        ew8[:, cs], start=True, stop=True)
                    if e < 6:   # GPSIMD can't read PSUM; split ACT/DVE
                        nc.scalar.copy(ewb[e][:, cs], eb_ps[:, :])
                    else:
                        nc.vector.tensor_scalar_add(ewb[e][:, cs],
                                                    eb_ps[:, :], 0.0)

            # ewz[e] = ew_e * z, produced during L0 and reused by all layers
            ewz = [act.tile([128, 2, NT], f16, name=f"ewz{e}", tag="ewz",
                            bufs=8) for e in range(E)]

            # ---- MoE layers ----
            layers = [
                (W0_d, None, 4, 4, DH),
                (W1_d, bias_sb[0], 6, 4, DH),
                (W2_d, bias_sb[1], 6, 4, DH),
                (WO_d, bias_sb[2], 6, 2, DM),
            ]
            xcur = None   # list of 4 act tiles (128, NT) for layers >= 1
            y_sb = None

            for li, (wd, b_sb, ktiles, douts, dout_dim) in enumerate(layers):
                # bufs=8: every expert's weights stay resident for the whole
                # layer (each tile is read by both token chunks; a smaller
                # ring deadlocks the chunk-outer loop order)
                w_tiles = []
                for e in range(E):
                    wt = wp.tile([128, ktiles, dout_dim], f16,
                                 name=f"w{li}_{e}", tag="w", bufs=8)
                    nc.sync.dma_start(wt, wd[e].rearrange("k p d -> p k d"))
                    w_tiles.append(wt)

                if li < 3:
                    xnext = [act.tile([128, NT], f16, name=f"x{li + 1}_{m}",
                                      tag="xp", bufs=8) for m in range(4)]
                else:
                    y_sb = [act.tile([128, NT], f16, name=f"y{m}",
                                     tag="xp", bufs=8) for m in range(2)]

                for c in range(CH):
                    cs = slice(c * CT, (c + 1) * CT)
                    if li == 0:
                        # packed bias+v matmul: one selector matmul + one DVE
                        # mul builds the [ew8; ew_e*v] moving operand. pk_ps
                        # is allocated BEFORE the accumulator psums so the
                        # 9-per-chunk PSUM ring phase only ever reuses slots
                        # whose elu already ran.
                        pk_ps = ps.tile([32, CT], f32, name="pk_ps", tag="ps")
                        nc.tensor.matmul(pk_ps[:, :], em32_sb[:, :],
                                         ew8[:, cs], start=True, stop=True)
                        pk_mv = xsp.tile([32, CT], f16, name="pk_mv",
                                         tag="pkm", bufs=2)
                        nc.vector.tensor_mul(pk_mv[:, :], pk_ps[:, :],
                                             v32_sb[:, cs])
                    psums = [ps.tile([128, CT], f32, name=f"mps{li}_{m}_{c}",
                                     tag="ps") for m in range(douts)]
                    for m in range(douts):
                        if li == 0:
                            nc.tensor.matmul(
                                psums[m][:, :],
                                pk0_sb[:, m * 128:(m + 1) * 128],
                                pk_mv[:, :], start=True, stop=False)
                        else:
                            nc.tensor.matmul(
                                psums[m][:, :],
                                b_sb[:, m * 128:(m + 1) * 128],
                                ew8[:, cs], start=True, stop=False)
                    for e in range(E):
                        # one xs tile per (expert, chunk) holding all the
                        # x/h k-tiles: fewer tile allocs = less semaphore
                        # teardown in the kernel epilogue
                        nkh = ktiles - 2
                        xs = xsp.tile([128, nkh, CT], f16, name="xs",
                                      tag="xs")
                        for kt in range(ktiles):
                            if kt < 2:   # z-part: cached ew-scaled z
                                if li == 0:
                                    nc.vector.tensor_mul(
                                        ewz[e][:, kt, cs], zk[kt][:, cs],
                                        ewb[e][:, cs])
                                mv = ewz[e][:, kt, cs]
                            else:
                                xsrc = (xc_sb[:, kt - 2, cs] if li == 0
                                        else xcur[kt - 2][:, cs])
                                nc.vector.tensor_mul(xs[:, kt - 2, :], xsrc,
                                                     ewb[e][:, cs])
                                mv = xs[:, kt - 2, :]
                            for m in range(douts):
                                nc.tensor.matmul(
                                    psums[m][:, :],
                                    w_tiles[e][:, kt,
                                               m * 128:(m + 1) * 128],
                                    mv,
                                    start=False,
                                    stop=(e == E - 1 and kt == ktiles - 1))
                    if li < 3:
                        for m in range(douts):
                            elu_p1(xnext[m][:, cs], psums[m][:, :])
                    else:
                        for m in range(douts):
                            nc.scalar.copy(y_sb[m][:, cs], psums[m][:, :])
                            nc.sync.dma_start(
                                yT_d[m * 128:(m + 1) * 128, cs],
                                y_sb[m][:, cs])
                if li < 3:
                    xcur = xnext

    nc.compile()
    return nc


def kernel(z, p_next, v_hip_next, x_curr,
           gw0, gb0, gw1, gb1, gw2, gb2,
           w0, b0, w1, b1, w2, b2, wo, bo):
    if "nc" not in _CACHE:
        _CACHE["nc"] = _build()
    nc = _CACHE["nc"]

    wdict = _prep_weights(
        np.asarray(gw0, np.float32), np.asarray(gb0, np.float32),
        np.asarray(gw1, np.float32), np.asarray(gb1, np.float32),
        np.asarray(gw2, np.float32), np.asarray(gb2, np.float32),
        np.asarray(w0, np.float32), np.asarray(b0, np.float32),
        np.asarray(w1, np.float32), np.asarray(b1, np.float32),
        np.asarray(w2, np.float32), np.asarray(b2, np.float32),
        np.asarray(wo, np.float32), np.asarray(bo, np.float32))

    in_maps = []
    for c in range(NCORES):
        m = _prep_core_inputs(np.asarray(z, np.float32),
                              np.asarray(p_next, np.float32),
                              np.asarray(v_hip_next, np.float32),
                              np.asarray(x_curr, np.float32), c)
        m.update(wdict)
        in_maps.append(m)

    res = bass_utils.run_bass_kernel_spmd(
        nc, in_maps, core_ids=list(range(NCORES)))

    out = np.empty((B, T, DM), np.float32)
    for c in range(NCORES):
        yT = res.results[c]["yT"].astype(np.float32)  # (DM, NT)
        out[c * BP:(c + 1) * BP] = yT.T.reshape(BP, T, DM)
    return out
